# revision 19
# baseline (speedup 1.0000x reference)
"""DeepSeek-V3 token-choice top-k router on 8 Trainium2 NeuronCores.

Strategy (per core, data-parallel over tokens; 1024 tokens/core):
  - Host: x and gate_w.T are scaled by 4096 and cast to a SINGLE fp16
    copy (the PE computes fp16 matmuls at ~bf16 internal precision, so
    hi/lo splitting buys no accuracy on HW — one pass is 3x cheaper).
    x is pre-transposed to d-major [128d, token] chunk layout.
  - Device per 128-token tile: 56 contraction chunks x 1 fp16 matmul
    (N=256 streaming columns) accumulating into a [128, 256] PSUM
    logits tile. x chunks are the stationary operand (LDWEIGHTS), the
    gate weight streams; FWL halves the LDWEIGHTS cost for fp16.
  - x streams per-tile (2 pieces/tile) on the SP HWDGE ring; the gate
    weight rides the ACT ring in 4 pieces so the first tile's matmuls
    start as soon as the first chunks land. ~40 warmup matmuls flip
    the PE HAM clock gate to 2.4 GHz during the initial DMA wait.
  - ACT: sigmoid(logits * 2^-24) PSUM->SBUF (scale undone for free).
  - DVE: bias add, per-group top-8 (`max`), top-4 group threshold via
    broadcast-AP mask multiply, masked top-8 + `max_index`, fp16
    one-hot score gathers (2x DVE rate), normalization.
"""

import numpy as np

N = 8192
D = 7168
E = 256
G = 8
EPG = E // G  # 32
TOPK_GROUP = 4
TOP_K = 8
SCALING = 2.5
N_CORES = 8
NPC = N // N_CORES  # 1024 tokens per core
P = 128
KC = D // P  # 56 contraction chunks
TT = NPC // P  # 8 token tiles per core
XPT = 2  # x pieces per tile
KPP = KC // XPT  # 28 chunks per x piece
GWP = 4  # gate-weight pieces
KPG = KC // GWP  # 14 chunks per gw piece
WARMUP_MM = 40
SX = 4096.0  # x scale (2^12)
SW = 4096.0  # w scale (2^12)

_CACHE = {}


def build_program():
    import concourse.bacc as bacc
    import concourse.mybir as mybir
    from concourse import tile

    nc = bacc.Bacc(
        "TRN2",
        target_bir_lowering=False,
        debug=False,
        enable_asserts=False,
        num_devices=N_CORES,
    )
    f16 = mybir.dt.float16
    f32 = mybir.dt.float32
    i16 = mybir.dt.int16
    i32 = mybir.dt.int32
    u32 = mybir.dt.uint32
    AF = mybir.ActivationFunctionType
    OP = mybir.AluOpType
    AX = mybir.AxisListType

    x_d = nc.dram_tensor("x2", [P, TT * KC * P], f16, kind="ExternalInput").ap()
    gw_d = nc.dram_tensor("gw2", [P, KC * E], f16, kind="ExternalInput").ap()
    bias_d = nc.dram_tensor("bias", [1, E], f32, kind="ExternalInput").ap()
    idx_d = nc.dram_tensor("idx", [NPC, TOP_K], i32, kind="ExternalOutput").ap()
    w_d = nc.dram_tensor("w", [NPC, TOP_K], f32, kind="ExternalOutput").ap()

    with tile.TileContext(nc) as tc:
        with (
            tc.tile_pool(name="const", bufs=1) as const_pool,
            tc.tile_pool(name="gw", bufs=1) as gw_pool,
            tc.tile_pool(name="xp", bufs=6) as x_pool,
            tc.tile_pool(name="plog", bufs=6, space="PSUM") as plog_pool,
            tc.tile_pool(name="junk", bufs=1, space="PSUM") as junk_pool,
            tc.tile_pool(name="work", bufs=5) as work_pool,
            tc.tile_pool(name="outs", bufs=5) as out_pool,
        ):
            # ---- gate weight pieces on the ACT HWDGE ring ----
            bias_sb = const_pool.tile([1, E], f32, name="biassb")
            nc.sync.dma_start(bias_sb[:], bias_d[:])
            gw_sb = []
            q = KPG * E
            for i in range(GWP):
                gw_sb.append(gw_pool.tile([P, q], f16, name=f"gw{i}"))
                nc.scalar.dma_start(gw_sb[i][:], gw_d[:, i * q : (i + 1) * q])
            gw_v = [g[:].rearrange("p (k e) -> p k e", k=KPG) for g in gw_sb]

            # ---- x pieces on the SP ring, in consumption order; the pool
            # + FIFO self-pace the prefetch.
            pieces = {}
            for t in range(TT):
                for i in range(XPT):
                    pc = x_pool.tile([P, KPP * P], f16, tag="xp", name=f"x{t}p{i}")
                    pieces[(t, i)] = pc
                    base = (t * KC + i * KPP) * P
                    nc.sync.dma_start(pc[:], x_d[:, base : base + KPP * P])

            # ---- constants ----
            iota_i = const_pool.tile([P, E], i32)
            nc.gpsimd.iota(iota_i[:], pattern=[[1, E]], base=0, channel_multiplier=0)
            iota_f = const_pool.tile([P, E], f32)
            nc.vector.tensor_copy(iota_f[:], iota_i[:])
            bias_rep = const_pool.tile([P, E], f32)
            nc.gpsimd.partition_broadcast(bias_rep[:], bias_sb[0:1, :])
            # ranks 1..8 (int16) — scatter payload for the rank map
            ranks16 = const_pool.tile([P, TOP_K], i16)
            nc.gpsimd.iota(ranks16[:], pattern=[[1, TOP_K]], base=1,
                           channel_multiplier=0)

            # ---- PE warmup: flip the HAM clock gate toward 8/8 during the
            # initial DMA wait.
            ij = iota_f[:].bitcast(f16)[:, 0:E]  # garbage-but-finite fp16
            junk = junk_pool.tile([P, 64], f32)
            for _ in range(WARMUP_MM):
                nc.tensor.matmul(junk[:], ij[:, 0:P], ij[:, 0:64], start=True,
                                 stop=True)

            def mm_tile(t, plog):
                for k in range(KC):
                    xp = pieces[(t, k // KPP)]
                    xk = xp[:, (k % KPP) * P : (k % KPP + 1) * P]
                    wk = gw_v[k // KPG][:, k % KPG, :]
                    nc.tensor.matmul(
                        plog[:], xk, wk, start=(k == 0), stop=(k == KC - 1),
                        skip_group_check=True,
                    )

            backlog = {}

            def routing_front(t, plog):
                scores = work_pool.tile([P, E], f32, tag="scores")
                nc.scalar.activation(
                    scores[:], plog[:], AF.Sigmoid, scale=1.0 / (SX * SW)
                )
                # fp16 copy (ACT engine) — scatter payload for the gather
                scores16 = work_pool.tile([P, E], f16, tag="scores16")
                nc.scalar.copy(scores16[:], scores[:])

                sfc = work_pool.tile([P, E], f32, tag="sfc")
                nc.vector.tensor_tensor(sfc[:], scores[:], bias_rep[:], op=OP.add)

                # per-group top-8 (need top-2 of each group of 32)
                gtops = work_pool.tile([P, G * 8], f32, tag="gtops")
                for g in range(G):
                    nc.vector.max(
                        gtops[:, g * 8 : (g + 1) * 8],
                        sfc[:, g * EPG : (g + 1) * EPG],
                    )
                gv = gtops[:].rearrange("p (g k) -> p g k", g=G)
                gs = work_pool.tile([P, G], f32, tag="gs")
                nc.vector.tensor_tensor(gs[:], gv[:, :, 0], gv[:, :, 1], op=OP.add)

                # top-4 groups -> mask
                gtop8 = work_pool.tile([P, 8], f32, tag="gtop8")
                nc.vector.max(gtop8[:], gs[:])
                gmask = work_pool.tile([P, G], f32, tag="gmask")
                nc.vector.tensor_scalar(
                    gmask[:], gs[:], gtop8[:, TOPK_GROUP - 1 : TOPK_GROUP], None,
                    op0=OP.is_ge,
                )

                # masked scores (broadcast the group mask over the 32 experts
                # of each group with a stride-0 AP)
                tmp = work_pool.tile([P, E], f32, tag="tmp")
                sfc_g = sfc[:].rearrange("p (g e) -> p g e", g=G)
                tmp_g = tmp[:].rearrange("p (g e) -> p g e", g=G)
                gmask_b = gmask[:].rearrange("p (g o) -> p g o", o=1).broadcast_to(
                    [P, G, EPG]
                )
                nc.vector.tensor_tensor(tmp_g, sfc_g, gmask_b, op=OP.mult)

                # top-8 values + indices
                vals = work_pool.tile([P, TOP_K], f32, tag="vals")
                nc.vector.max(vals[:], tmp[:])
                idxu = work_pool.tile([P, TOP_K], u32, tag="idxu")
                nc.vector.max_index(idxu[:], vals[:], tmp[:])

                # idx output is ready now -- ship it while the gather runs
                nc.scalar.dma_start(
                    idx_d[t * P : (t + 1) * P, :], idxu[:].bitcast(i32)
                )

                # gather scores[idx] via gpsimd local_scatters, all on the
                # Pool queue (no DVE round-trips):
                #   1) rank_map[e] = slot+1 at selected experts (0 elsewhere)
                #   2) targ = rank_map - 1 (-1 = unselected -> ignored)
                #   3) w16[slot] = scores16[e] scattered by targ
                idx16 = work_pool.tile([P, TOP_K], i16, tag="idx16")
                nc.vector.tensor_copy(idx16[:], idxu[:])
                rank_map = work_pool.tile([P, E], i16, tag="rankmap")
                nc.gpsimd.local_scatter(
                    rank_map[:], ranks16[:], idx16[:],
                    channels=P, num_elems=E, num_idxs=TOP_K,
                )
                targ = work_pool.tile([P, E], i16, tag="targ")
                nc.gpsimd.tensor_scalar(
                    targ[:], rank_map[:], 1, None, op0=OP.subtract
                )
                w16 = work_pool.tile([P, TOP_K], f16, tag="w16")
                nc.gpsimd.local_scatter(
                    w16[:], scores16[:], targ[:],
                    channels=P, num_elems=TOP_K, num_idxs=E,
                )
                backlog[t] = w16

            def routing_back(t):
                w16 = backlog.pop(t)
                w8 = out_pool.tile([P, TOP_K], f32, tag="w8")
                nc.vector.tensor_copy(w8[:], w16[:])

                # normalize + scale (wsum > 0 always: sigmoid outputs)
                wsum = work_pool.tile([P, 1], f32, tag="wsum")
                nc.vector.reduce_sum(wsum[:], w8[:], axis=AX.X)
                wrec = work_pool.tile([P, 1], f32, tag="wrec")
                nc.vector.reciprocal(wrec[:], wsum[:])
                w_out = out_pool.tile([P, TOP_K], f32, tag="wout")
                nc.vector.tensor_scalar(
                    w_out[:], w8[:], wrec[:, 0:1], float(SCALING),
                    op0=OP.mult, op1=OP.mult,
                )
                nc.scalar.dma_start(w_d[t * P : (t + 1) * P, :], w_out[:])

            for t in range(TT):
                plog = plog_pool.tile([P, E], f32, tag="plog", name=f"plog{t}")
                mm_tile(t, plog)
                routing_front(t, plog)
                if t > 0:
                    routing_back(t - 1)
            routing_back(TT - 1)

    nc.compile()
    return nc


def _get_nc(**kw):
    key = tuple(sorted(kw.items()))
    if key not in _CACHE:
        _CACHE[key] = build_program(**kw)
    return _CACHE[key]


def _pack_x(xh):
    # [8192, 7168] fp16 -> [8 cores, 128, TT*KC*128]; per core, partition p
    # holds d = k*128+p; free index = t*(KC*128) + k*128 + token.
    b = xh.reshape(N_CORES, TT, P, KC, P).transpose(0, 4, 1, 3, 2)
    return np.ascontiguousarray(b).reshape(N_CORES, P, TT * KC * P)


def _prep_inputs(x, gate_w, bias):
    xh = (x * np.float32(SX)).astype(np.float16)
    xp = _pack_x(xh)

    ws = np.ascontiguousarray(gate_w.T) * np.float32(SW)  # [D, E]
    wh = ws.astype(np.float16)
    gw2 = np.ascontiguousarray(
        wh.reshape(KC, P, E).transpose(1, 0, 2)
    ).reshape(P, KC * E)
    bias2d = np.ascontiguousarray(bias.reshape(1, E))
    return xp, gw2, bias2d


def _run(x, gate_w, bias, trace=False, **build_kw):
    from concourse.bass_utils import run_bass_kernel_spmd

    x = np.ascontiguousarray(np.asarray(x, dtype=np.float32))
    gate_w = np.ascontiguousarray(np.asarray(gate_w, dtype=np.float32))
    bias = np.ascontiguousarray(np.asarray(bias, dtype=np.float32))
    nc = _get_nc(**build_kw)
    xp, gw2, bias2d = _prep_inputs(x, gate_w, bias)
    in_maps = [
        {"x2": xp[c], "gw2": gw2, "bias": bias2d} for c in range(N_CORES)
    ]
    res = run_bass_kernel_spmd(nc, in_maps, core_ids=list(range(N_CORES)), trace=trace)
    idx = np.concatenate([res.results[c]["idx"] for c in range(N_CORES)], axis=0)
    w = np.concatenate([res.results[c]["w"] for c in range(N_CORES)], axis=0)
    return (idx.astype(np.int32), w.astype(np.float32)), res


def kernel(x, gate_w, bias):
    (idx, w), _ = _run(x, gate_w, bias)
    return idx, w


# revision 21
# speedup vs baseline: 1.3313x; 1.3313x over previous
"""DeepSeek-V3 token-choice top-k router on 8 Trainium2 NeuronCores.

Strategy (per core, data-parallel over tokens; 1024 tokens/core):
  - Host: x and gate_w.T are scaled by 4096 and cast to a SINGLE fp16
    copy (the PE computes fp16 matmuls at ~bf16 internal precision, so
    hi/lo splitting buys no accuracy on HW — one pass is 3x cheaper).
    x is pre-transposed to d-major [128d, token] chunk layout.
  - Device per 128-token tile: 56 contraction chunks x 1 fp16 matmul
    (N=256 streaming columns) accumulating into a [128, 256] PSUM
    logits tile. x chunks are the stationary operand (LDWEIGHTS), the
    gate weight streams; FWL halves the LDWEIGHTS cost for fp16.
  - x streams per-tile (2 pieces/tile) on the SP HWDGE ring; the gate
    weight rides the ACT ring in 4 pieces so the first tile's matmuls
    start as soon as the first chunks land. ~40 warmup matmuls flip
    the PE HAM clock gate to 2.4 GHz during the initial DMA wait.
  - ACT: sigmoid(logits * 2^-24) PSUM->SBUF (scale undone for free).
  - DVE: bias add, per-group top-8 (`max`), top-4 group threshold via
    broadcast-AP mask multiply, masked top-8 + `max_index`, fp16
    one-hot score gathers (2x DVE rate), normalization.
"""

import numpy as np

N = 8192
D = 7168
E = 256
G = 8
EPG = E // G  # 32
TOPK_GROUP = 4
TOP_K = 8
SCALING = 2.5
N_CORES = 8
NPC = N // N_CORES  # 1024 tokens per core
P = 128
KC = D // P  # 56 contraction chunks
TT = NPC // P  # 8 token tiles per core
XPT = 2  # x pieces per tile
KPP = KC // XPT  # 28 chunks per x piece
GWP = 4  # gate-weight pieces
KPG = KC // GWP  # 14 chunks per gw piece
WARMUP_MM = 40
SX = 4096.0  # x scale (2^12)
SW = 4096.0  # w scale (2^12)

_CACHE = {}


def build_program():
    import concourse.bacc as bacc
    import concourse.mybir as mybir
    from concourse import tile

    nc = bacc.Bacc(
        "TRN2",
        target_bir_lowering=False,
        debug=False,
        enable_asserts=False,
        num_devices=N_CORES,
    )
    f16 = mybir.dt.float16
    f32 = mybir.dt.float32
    i16 = mybir.dt.int16
    i32 = mybir.dt.int32
    u32 = mybir.dt.uint32
    AF = mybir.ActivationFunctionType
    OP = mybir.AluOpType
    AX = mybir.AxisListType

    x_d = nc.dram_tensor("x2", [P, TT * KC * P], f16, kind="ExternalInput").ap()
    gw_d = nc.dram_tensor("gw2", [P, KC * E], f16, kind="ExternalInput").ap()
    bias_d = nc.dram_tensor("bias", [1, E], f32, kind="ExternalInput").ap()
    idx_d = nc.dram_tensor("idx", [NPC, TOP_K], i32, kind="ExternalOutput").ap()
    w_d = nc.dram_tensor("w", [NPC, TOP_K], f32, kind="ExternalOutput").ap()

    with tile.TileContext(nc) as tc:
        with (
            tc.tile_pool(name="const", bufs=1) as const_pool,
            tc.tile_pool(name="gw", bufs=1) as gw_pool,
            tc.tile_pool(name="xp", bufs=6) as x_pool,
            tc.tile_pool(name="plog", bufs=6, space="PSUM") as plog_pool,
            tc.tile_pool(name="junk", bufs=1, space="PSUM") as junk_pool,
            tc.tile_pool(name="work", bufs=5) as work_pool,
            tc.tile_pool(name="outs", bufs=5) as out_pool,
        ):
            # ---- gate weight pieces on the ACT HWDGE ring ----
            bias_sb = const_pool.tile([1, E], f32, name="biassb")
            nc.sync.dma_start(bias_sb[:], bias_d[:])
            gw_sb = []
            q = KPG * E
            for i in range(GWP):
                gw_sb.append(gw_pool.tile([P, q], f16, name=f"gw{i}"))
                nc.scalar.dma_start(gw_sb[i][:], gw_d[:, i * q : (i + 1) * q])
            gw_v = [g[:].rearrange("p (k e) -> p k e", k=KPG) for g in gw_sb]

            # ---- x pieces on the SP ring, in consumption order; the pool
            # + FIFO self-pace the prefetch.
            pieces = {}
            for t in range(TT):
                for i in range(XPT):
                    pc = x_pool.tile([P, KPP * P], f16, tag="xp", name=f"x{t}p{i}")
                    pieces[(t, i)] = pc
                    base = (t * KC + i * KPP) * P
                    nc.sync.dma_start(pc[:], x_d[:, base : base + KPP * P])

            # ---- constants ----
            iota_i = const_pool.tile([P, E], i32)
            nc.gpsimd.iota(iota_i[:], pattern=[[1, E]], base=0, channel_multiplier=0)
            iota_f = const_pool.tile([P, E], f32)
            nc.vector.tensor_copy(iota_f[:], iota_i[:])
            bias_rep = const_pool.tile([P, E], f32)
            nc.gpsimd.partition_broadcast(bias_rep[:], bias_sb[0:1, :])
            # ranks 1..8 (int16) — scatter payload for the rank map
            ranks16 = const_pool.tile([P, TOP_K], i16)
            nc.gpsimd.iota(ranks16[:], pattern=[[1, TOP_K]], base=1,
                           channel_multiplier=0)

            # ---- PE warmup: flip the HAM clock gate toward 8/8 during the
            # initial DMA wait.
            ij = iota_f[:].bitcast(f16)[:, 0:E]  # garbage-but-finite fp16
            junk = junk_pool.tile([P, 64], f32)
            for _ in range(WARMUP_MM):
                nc.tensor.matmul(junk[:], ij[:, 0:P], ij[:, 0:64], start=True,
                                 stop=True)

            def mm_tile(t, plog):
                for k in range(KC):
                    xp = pieces[(t, k // KPP)]
                    xk = xp[:, (k % KPP) * P : (k % KPP + 1) * P]
                    wk = gw_v[k // KPG][:, k % KPG, :]
                    nc.tensor.matmul(
                        plog[:], xk, wk, start=(k == 0), stop=(k == KC - 1),
                        skip_group_check=True,
                    )

            backlog = {}

            def routing_front(t, plog):
                scores = work_pool.tile([P, E], f32, tag="scores")
                nc.scalar.activation(
                    scores[:], plog[:], AF.Sigmoid, scale=1.0 / (SX * SW)
                )
                # fp16 copy (ACT engine) — scatter payload for the gather
                scores16 = work_pool.tile([P, E], f16, tag="scores16")
                nc.scalar.copy(scores16[:], scores[:])

                sfc = work_pool.tile([P, E], f32, tag="sfc")
                nc.vector.tensor_tensor(sfc[:], scores[:], bias_rep[:], op=OP.add)

                # per-group top-8 (need top-2 of each group of 32)
                gtops = work_pool.tile([P, G * 8], f32, tag="gtops")
                for g in range(G):
                    nc.vector.max(
                        gtops[:, g * 8 : (g + 1) * 8],
                        sfc[:, g * EPG : (g + 1) * EPG],
                    )
                gv = gtops[:].rearrange("p (g k) -> p g k", g=G)
                gs = work_pool.tile([P, G], f32, tag="gs")
                nc.vector.tensor_tensor(gs[:], gv[:, :, 0], gv[:, :, 1], op=OP.add)

                # top-4 groups -> mask
                gtop8 = work_pool.tile([P, 8], f32, tag="gtop8")
                nc.vector.max(gtop8[:], gs[:])
                gmask = work_pool.tile([P, G], f32, tag="gmask")
                nc.vector.tensor_scalar(
                    gmask[:], gs[:], gtop8[:, TOPK_GROUP - 1 : TOPK_GROUP], None,
                    op0=OP.is_ge,
                )

                # masked scores (broadcast the group mask over the 32 experts
                # of each group with a stride-0 AP)
                tmp = work_pool.tile([P, E], f32, tag="tmp")
                sfc_g = sfc[:].rearrange("p (g e) -> p g e", g=G)
                tmp_g = tmp[:].rearrange("p (g e) -> p g e", g=G)
                gmask_b = gmask[:].rearrange("p (g o) -> p g o", o=1).broadcast_to(
                    [P, G, EPG]
                )
                nc.vector.tensor_tensor(tmp_g, sfc_g, gmask_b, op=OP.mult)

                # top-8 values + indices
                vals = work_pool.tile([P, TOP_K], f32, tag="vals")
                nc.vector.max(vals[:], tmp[:])
                idxu = work_pool.tile([P, TOP_K], u32, tag="idxu")
                nc.vector.max_index(idxu[:], vals[:], tmp[:])

                # idx output is ready now -- ship it while the gather runs
                nc.scalar.dma_start(
                    idx_d[t * P : (t + 1) * P, :], idxu[:].bitcast(i32)
                )

                # gather scores[idx] via gpsimd local_scatters, all on the
                # Pool queue (no DVE round-trips):
                #   1) rank_map[e] = slot+1 at selected experts (0 elsewhere)
                #   2) targ = rank_map - 1 (-1 = unselected -> ignored)
                #   3) w16[slot] = scores16[e] scattered by targ
                idx16 = work_pool.tile([P, TOP_K], i16, tag="idx16")
                nc.vector.tensor_copy(idx16[:], idxu[:])
                rank_map = work_pool.tile([P, E], i16, tag="rankmap")
                nc.gpsimd.local_scatter(
                    rank_map[:], ranks16[:], idx16[:],
                    channels=P, num_elems=E, num_idxs=TOP_K,
                )
                backlog[t] = (rank_map, scores16)

            def routing_mid(t):
                rank_map, scores16 = backlog.pop(t)
                targ = work_pool.tile([P, E], i16, tag="targ")
                nc.vector.tensor_scalar(
                    targ[:], rank_map[:], 1, None, op0=OP.subtract
                )
                w16 = work_pool.tile([P, TOP_K], f16, tag="w16")
                nc.gpsimd.local_scatter(
                    w16[:], scores16[:], targ[:],
                    channels=P, num_elems=TOP_K, num_idxs=E,
                )
                backlog[(t, "w")] = w16

            def routing_back(t):
                w16 = backlog.pop((t, "w"))
                w8 = out_pool.tile([P, TOP_K], f32, tag="w8")
                nc.vector.tensor_copy(w8[:], w16[:])

                # normalize + scale (wsum > 0 always: sigmoid outputs)
                wsum = work_pool.tile([P, 1], f32, tag="wsum")
                nc.vector.reduce_sum(wsum[:], w8[:], axis=AX.X)
                wrec = work_pool.tile([P, 1], f32, tag="wrec")
                nc.vector.reciprocal(wrec[:], wsum[:])
                w_out = out_pool.tile([P, TOP_K], f32, tag="wout")
                nc.vector.tensor_scalar(
                    w_out[:], w8[:], wrec[:, 0:1], float(SCALING),
                    op0=OP.mult, op1=OP.mult,
                )
                nc.scalar.dma_start(w_d[t * P : (t + 1) * P, :], w_out[:])

            for t in range(TT):
                plog = plog_pool.tile([P, E], f32, tag="plog", name=f"plog{t}")
                mm_tile(t, plog)
                routing_front(t, plog)
                if t >= 1:
                    routing_mid(t - 1)
                if t >= 2:
                    routing_back(t - 2)
            routing_mid(TT - 1)
            routing_back(TT - 2)
            routing_back(TT - 1)

    nc.compile()
    return nc


def _get_nc(**kw):
    key = tuple(sorted(kw.items()))
    if key not in _CACHE:
        _CACHE[key] = build_program(**kw)
    return _CACHE[key]


def _pack_x(xh):
    # [8192, 7168] fp16 -> [8 cores, 128, TT*KC*128]; per core, partition p
    # holds d = k*128+p; free index = t*(KC*128) + k*128 + token.
    b = xh.reshape(N_CORES, TT, P, KC, P).transpose(0, 4, 1, 3, 2)
    return np.ascontiguousarray(b).reshape(N_CORES, P, TT * KC * P)


def _prep_inputs(x, gate_w, bias):
    xh = (x * np.float32(SX)).astype(np.float16)
    xp = _pack_x(xh)

    ws = np.ascontiguousarray(gate_w.T) * np.float32(SW)  # [D, E]
    wh = ws.astype(np.float16)
    gw2 = np.ascontiguousarray(
        wh.reshape(KC, P, E).transpose(1, 0, 2)
    ).reshape(P, KC * E)
    bias2d = np.ascontiguousarray(bias.reshape(1, E))
    return xp, gw2, bias2d


def _run(x, gate_w, bias, trace=False, **build_kw):
    from concourse.bass_utils import run_bass_kernel_spmd

    x = np.ascontiguousarray(np.asarray(x, dtype=np.float32))
    gate_w = np.ascontiguousarray(np.asarray(gate_w, dtype=np.float32))
    bias = np.ascontiguousarray(np.asarray(bias, dtype=np.float32))
    nc = _get_nc(**build_kw)
    xp, gw2, bias2d = _prep_inputs(x, gate_w, bias)
    in_maps = [
        {"x2": xp[c], "gw2": gw2, "bias": bias2d} for c in range(N_CORES)
    ]
    res = run_bass_kernel_spmd(nc, in_maps, core_ids=list(range(N_CORES)), trace=trace)
    idx = np.concatenate([res.results[c]["idx"] for c in range(N_CORES)], axis=0)
    w = np.concatenate([res.results[c]["w"] for c in range(N_CORES)], axis=0)
    return (idx.astype(np.int32), w.astype(np.float32)), res


def kernel(x, gate_w, bias):
    (idx, w), _ = _run(x, gate_w, bias)
    return idx, w


# revision 24
# speedup vs baseline: 1.4261x; 1.0712x over previous
"""DeepSeek-V3 token-choice top-k router on 8 Trainium2 NeuronCores.

Strategy (per core, data-parallel over tokens; 1024 tokens/core):
  - Host: x and gate_w.T are scaled by 4096 and cast to a SINGLE fp16
    copy (the PE computes fp16 matmuls at ~bf16 internal precision, so
    hi/lo splitting buys no accuracy on HW — one pass is 3x cheaper).
    x is pre-transposed to d-major [128d, token] chunk layout.
  - Device per 128-token tile: 56 contraction chunks x 1 fp16 matmul
    (N=256 streaming columns) accumulating into a [128, 256] PSUM
    logits tile. x chunks are the stationary operand (LDWEIGHTS), the
    gate weight streams; FWL halves the LDWEIGHTS cost for fp16.
  - x streams per-tile (2 pieces/tile) on the SP HWDGE ring; the gate
    weight rides the ACT ring in 4 pieces so the first tile's matmuls
    start as soon as the first chunks land. ~40 warmup matmuls flip
    the PE HAM clock gate to 2.4 GHz during the initial DMA wait.
  - ACT: sigmoid(logits * 2^-24) PSUM->SBUF (scale undone for free).
  - DVE: bias add, per-group top-8 (`max`), top-4 group threshold via
    broadcast-AP mask multiply, masked top-8 + `max_index`, fp16
    one-hot score gathers (2x DVE rate), normalization.
"""

import numpy as np

N = 8192
D = 7168
E = 256
G = 8
EPG = E // G  # 32
TOPK_GROUP = 4
TOP_K = 8
SCALING = 2.5
N_CORES = 8
NPC = N // N_CORES  # 1024 tokens per core
P = 128
KC = D // P  # 56 contraction chunks
TT = NPC // P  # 8 token tiles per core
XPT = 2  # x pieces per tile
KPP = KC // XPT  # 28 chunks per x piece
GWP = 4  # gate-weight pieces
KPG = KC // GWP  # 14 chunks per gw piece
WARMUP_MM = 40
SX = 4096.0  # x scale (2^12)
SW = 4096.0  # w scale (2^12)

_CACHE = {}


def build_program():
    import concourse.bacc as bacc
    import concourse.mybir as mybir
    from concourse import tile

    nc = bacc.Bacc(
        "TRN2",
        target_bir_lowering=False,
        debug=False,
        enable_asserts=False,
        num_devices=N_CORES,
    )
    f16 = mybir.dt.float16
    f32 = mybir.dt.float32
    i16 = mybir.dt.int16
    i32 = mybir.dt.int32
    u32 = mybir.dt.uint32
    AF = mybir.ActivationFunctionType
    OP = mybir.AluOpType
    AX = mybir.AxisListType

    x_d = nc.dram_tensor("x2", [P, TT * KC * P], f16, kind="ExternalInput").ap()
    gw_d = nc.dram_tensor("gw2", [P, KC * E], f16, kind="ExternalInput").ap()
    bias_d = nc.dram_tensor("bias", [1, E], f32, kind="ExternalInput").ap()
    idx_d = nc.dram_tensor("idx", [NPC, TOP_K], i32, kind="ExternalOutput").ap()
    w_d = nc.dram_tensor("w", [NPC, TOP_K], f32, kind="ExternalOutput").ap()

    with tile.TileContext(nc) as tc:
        with (
            tc.tile_pool(name="const", bufs=1) as const_pool,
            tc.tile_pool(name="gw", bufs=1) as gw_pool,
            tc.tile_pool(name="xp", bufs=6) as x_pool,
            tc.tile_pool(name="plog", bufs=6, space="PSUM") as plog_pool,
            tc.tile_pool(name="junk", bufs=1, space="PSUM") as junk_pool,
            tc.tile_pool(name="work", bufs=5) as work_pool,
            tc.tile_pool(name="outs", bufs=5) as out_pool,
        ):
            # ---- gate weight pieces on the ACT HWDGE ring ----
            bias_sb = const_pool.tile([1, E], f32, name="biassb")
            nc.sync.dma_start(bias_sb[:], bias_d[:])
            gw_sb = []
            q = KPG * E
            for i in range(GWP):
                gw_sb.append(gw_pool.tile([P, q], f16, name=f"gw{i}"))
                nc.scalar.dma_start(gw_sb[i][:], gw_d[:, i * q : (i + 1) * q])
            gw_v = [g[:].rearrange("p (k e) -> p k e", k=KPG) for g in gw_sb]

            # ---- x pieces on the SP ring, in consumption order; the pool
            # + FIFO self-pace the prefetch.
            pieces = {}
            for t in range(TT):
                for i in range(XPT):
                    pc = x_pool.tile([P, KPP * P], f16, tag="xp", name=f"x{t}p{i}")
                    pieces[(t, i)] = pc
                    base = (t * KC + i * KPP) * P
                    nc.sync.dma_start(pc[:], x_d[:, base : base + KPP * P])

            # ---- constants ----
            iota_i = const_pool.tile([P, E], i32)
            nc.gpsimd.iota(iota_i[:], pattern=[[1, E]], base=0, channel_multiplier=0)
            iota_f = const_pool.tile([P, E], f32)
            nc.vector.tensor_copy(iota_f[:], iota_i[:])
            bias_rep = const_pool.tile([P, E], f32)
            nc.gpsimd.partition_broadcast(bias_rep[:], bias_sb[0:1, :])
            # ranks 1..8 (int16) — scatter payload for the rank map
            ranks16 = const_pool.tile([P, TOP_K], i16)
            nc.gpsimd.iota(ranks16[:], pattern=[[1, TOP_K]], base=1,
                           channel_multiplier=0)

            # ---- PE warmup: flip the HAM clock gate toward 8/8 during the
            # initial DMA wait.
            ij = iota_f[:].bitcast(f16)[:, 0:E]  # garbage-but-finite fp16
            junk = junk_pool.tile([P, 64], f32)
            for _ in range(WARMUP_MM):
                nc.tensor.matmul(junk[:], ij[:, 0:P], ij[:, 0:64], start=True,
                                 stop=True)

            def mm_tile(t, plog):
                for k in range(KC):
                    xp = pieces[(t, k // KPP)]
                    xk = xp[:, (k % KPP) * P : (k % KPP + 1) * P]
                    wk = gw_v[k // KPG][:, k % KPG, :]
                    nc.tensor.matmul(
                        plog[:], xk, wk, start=(k == 0), stop=(k == KC - 1),
                        skip_group_check=True,
                    )

            backlog = {}

            def routing_front(t, plog):
                scores = work_pool.tile([P, E], f32, tag="scores")
                nc.scalar.activation(
                    scores[:], plog[:], AF.Sigmoid, scale=1.0 / (SX * SW)
                )
                # fp16 copy (ACT engine) — scatter payload for the gather
                scores16 = work_pool.tile([P, E], f16, tag="scores16")
                nc.scalar.copy(scores16[:], scores[:])

                sfc = work_pool.tile([P, E], f32, tag="sfc")
                nc.vector.tensor_tensor(sfc[:], scores[:], bias_rep[:], op=OP.add)

                # per-group top-8 (need top-2 of each group of 32)
                gtops = work_pool.tile([P, G * 8], f32, tag="gtops")
                for g in range(G):
                    nc.vector.max(
                        gtops[:, g * 8 : (g + 1) * 8],
                        sfc[:, g * EPG : (g + 1) * EPG],
                    )
                gv = gtops[:].rearrange("p (g k) -> p g k", g=G)
                gs = work_pool.tile([P, G], f32, tag="gs")
                nc.vector.tensor_tensor(gs[:], gv[:, :, 0], gv[:, :, 1], op=OP.add)

                # top-4 groups -> mask
                gtop8 = work_pool.tile([P, 8], f32, tag="gtop8")
                nc.vector.max(gtop8[:], gs[:])
                gmask = work_pool.tile([P, G], f32, tag="gmask")
                nc.vector.tensor_scalar(
                    gmask[:], gs[:], gtop8[:, TOPK_GROUP - 1 : TOPK_GROUP], None,
                    op0=OP.is_ge,
                )

                # masked scores (broadcast the group mask over the 32 experts
                # of each group with a stride-0 AP)
                tmp = work_pool.tile([P, E], f32, tag="tmp")
                sfc_g = sfc[:].rearrange("p (g e) -> p g e", g=G)
                tmp_g = tmp[:].rearrange("p (g e) -> p g e", g=G)
                gmask_b = gmask[:].rearrange("p (g o) -> p g o", o=1).broadcast_to(
                    [P, G, EPG]
                )
                nc.vector.tensor_tensor(tmp_g, sfc_g, gmask_b, op=OP.mult)

                # top-8 values + indices
                vals = work_pool.tile([P, TOP_K], f32, tag="vals")
                nc.vector.max(vals[:], tmp[:])
                idxu = work_pool.tile([P, TOP_K], u32, tag="idxu")
                nc.vector.max_index(idxu[:], vals[:], tmp[:])

                # idx output is ready now -- ship it while the gather runs
                nc.sync.dma_start(
                    idx_d[t * P : (t + 1) * P, :], idxu[:].bitcast(i32)
                )

                # gather scores[idx] via gpsimd local_scatters, all on the
                # Pool queue (no DVE round-trips):
                #   1) rank_map[e] = slot+1 at selected experts (0 elsewhere)
                #   2) targ = rank_map - 1 (-1 = unselected -> ignored)
                #   3) w16[slot] = scores16[e] scattered by targ
                idx16 = work_pool.tile([P, TOP_K], i16, tag="idx16")
                nc.vector.tensor_copy(idx16[:], idxu[:])
                rank_map = work_pool.tile([P, E], i16, tag="rankmap")
                nc.gpsimd.local_scatter(
                    rank_map[:], ranks16[:], idx16[:],
                    channels=P, num_elems=E, num_idxs=TOP_K,
                )
                backlog[t] = (rank_map, scores16)

            def routing_mid(t):
                rank_map, scores16 = backlog.pop(t)
                targ = work_pool.tile([P, E], i16, tag="targ")
                nc.vector.tensor_scalar(
                    targ[:], rank_map[:], 1, None, op0=OP.subtract
                )
                w16 = work_pool.tile([P, TOP_K], f16, tag="w16")
                nc.gpsimd.local_scatter(
                    w16[:], scores16[:], targ[:],
                    channels=P, num_elems=TOP_K, num_idxs=E,
                )
                backlog[(t, "w")] = w16

            def routing_back(t):
                w16 = backlog.pop((t, "w"))
                w8 = out_pool.tile([P, TOP_K], f32, tag="w8")
                nc.vector.tensor_copy(w8[:], w16[:])

                # normalize + scale (wsum > 0 always: sigmoid outputs)
                wsum = work_pool.tile([P, 1], f32, tag="wsum")
                nc.vector.reduce_sum(wsum[:], w8[:], axis=AX.X)
                wrec = work_pool.tile([P, 1], f32, tag="wrec")
                nc.vector.reciprocal(wrec[:], wsum[:])
                w_out = out_pool.tile([P, TOP_K], f32, tag="wout")
                nc.vector.tensor_scalar(
                    w_out[:], w8[:], wrec[:, 0:1], float(SCALING),
                    op0=OP.mult, op1=OP.mult,
                )
                # w/idx DMA triggers ride the SYNC queue: a trigger waiting
                # here must not block the next tile's sigmoid (scalar queue)
                nc.sync.dma_start(w_d[t * P : (t + 1) * P, :], w_out[:])

            for t in range(TT):
                plog = plog_pool.tile([P, E], f32, tag="plog", name=f"plog{t}")
                mm_tile(t, plog)
                routing_front(t, plog)
                if t >= 1:
                    routing_mid(t - 1)
                if t >= 2:
                    routing_back(t - 2)
            routing_mid(TT - 1)
            routing_back(TT - 2)
            routing_back(TT - 1)

    nc.compile()
    return nc


def _get_nc(**kw):
    key = tuple(sorted(kw.items()))
    if key not in _CACHE:
        _CACHE[key] = build_program(**kw)
    return _CACHE[key]


def _pack_x(xh):
    # [8192, 7168] fp16 -> [8 cores, 128, TT*KC*128]; per core, partition p
    # holds d = k*128+p; free index = t*(KC*128) + k*128 + token.
    b = xh.reshape(N_CORES, TT, P, KC, P).transpose(0, 4, 1, 3, 2)
    return np.ascontiguousarray(b).reshape(N_CORES, P, TT * KC * P)


def _prep_inputs(x, gate_w, bias):
    xh = (x * np.float32(SX)).astype(np.float16)
    xp = _pack_x(xh)

    ws = np.ascontiguousarray(gate_w.T) * np.float32(SW)  # [D, E]
    wh = ws.astype(np.float16)
    gw2 = np.ascontiguousarray(
        wh.reshape(KC, P, E).transpose(1, 0, 2)
    ).reshape(P, KC * E)
    bias2d = np.ascontiguousarray(bias.reshape(1, E))
    return xp, gw2, bias2d


def _run(x, gate_w, bias, trace=False, **build_kw):
    from concourse.bass_utils import run_bass_kernel_spmd

    x = np.ascontiguousarray(np.asarray(x, dtype=np.float32))
    gate_w = np.ascontiguousarray(np.asarray(gate_w, dtype=np.float32))
    bias = np.ascontiguousarray(np.asarray(bias, dtype=np.float32))
    nc = _get_nc(**build_kw)
    xp, gw2, bias2d = _prep_inputs(x, gate_w, bias)
    in_maps = [
        {"x2": xp[c], "gw2": gw2, "bias": bias2d} for c in range(N_CORES)
    ]
    res = run_bass_kernel_spmd(nc, in_maps, core_ids=list(range(N_CORES)), trace=trace)
    idx = np.concatenate([res.results[c]["idx"] for c in range(N_CORES)], axis=0)
    w = np.concatenate([res.results[c]["w"] for c in range(N_CORES)], axis=0)
    return (idx.astype(np.int32), w.astype(np.float32)), res


def kernel(x, gate_w, bias):
    (idx, w), _ = _run(x, gate_w, bias)
    return idx, w
